# revision 22
# baseline (speedup 1.0000x reference)
"""Bahdanau attention Trainium2 kernel.

  keys_proj = values @ W1 + b1            # (B, T, U)
  query_proj = query @ W2 + b2            # (B, 1, U)
  score = tanh(keys_proj + query_proj) @ V + bv   # (B, T, 1)
  aw = softmax(score, axis=1)             # (B, T, 1)
  ctx = sum(aw * values, axis=1)          # (B, D)

Sharding: data-parallel over batch, 4 batches per core on 8 cores. Each core
is fully independent (no collectives).

Per-core dataflow (streaming over t-chunks with an online softmax, so values
is read from HBM exactly once and never retained beyond the current chunk):
  - values chunk loaded naturally [t=128, d=2048] (contiguous DMA).
  - PE-transpose 128x128 blocks -> vT [d=128, t=TCH] (f32r, 1.5 cyc/row).
  - kp[u=128, t] += W1[d,u-tile].T @ vT (PSUM accumulate over 16 d-tiles),
    float32r so the PE runs at full rate with 4-byte operands (moving
    dim >= 256).  float32r is a reduced-mantissa fp32 PE mode; end-to-end
    error vs the fp32 reference measures ~2.4e-4.
  - ScalarE: th = tanh(kp + (q @ W2 + b2 + b1)[u]) with per-partition bias,
    PSUM -> SBUF in one activation op.
  - score[1, t] += V[u-tile].T @ th  (PSUM accumulate over 8 u-tiles).
  - online softmax state update on partition 0 (negm = -running max, s =
    running sum): DVE reduce max (negated) / min / sub, ACT exp of the
    chunk with fused accum_out sum.  bv is dropped: softmax is
    shift-invariant and both outputs depend on score only through softmax.
  - context update ctx[1, d] = alpha * ctx + w_chunk @ values_chunk: chunk
    weights spread to [t=128, TT] columns via K=1 matmuls against ones,
    then TT accumulating matmuls against the still-resident natural chunk
    tiles, and a DVE rescale-add.
  - batch epilogue: ctx/s DMA'd out, attention weights recomputed from the
    stored score row as exp(score - max)/s.
"""

import os
from contextlib import ExitStack

import numpy as np

import concourse.bass as bass
import concourse.mybir as mybir
import concourse.tile as tile
from concourse.bass_utils import run_bass_kernel_spmd
from concourse.masks import make_identity

B, T, D, U = 32, 1024, 2048, 1024
NCORES = 8
BPC = B // NCORES  # batches per core
P = 128
DT = D // P  # 16 d-tiles
UT = U // P  # 8 u-tiles
TCH = int(os.environ.get("BAHDANAU_TCH", "256"))  # main-matmul moving dim; >=256 for f32r full rate
NCH = T // TCH  # chunks per batch
TT = TCH // P  # t-subtiles per chunk
DCH = 512  # d-chunk of the context matmul
NAT_BUFS = int(os.environ.get("BAHDANAU_NAT_BUFS", "8"))
VT_BUFS = int(os.environ.get("BAHDANAU_VT_BUFS", "2"))
TH_BUFS = int(os.environ.get("BAHDANAU_TH_BUFS", "2"))
KP_BUFS = int(os.environ.get("BAHDANAU_KP_BUFS", "2"))
COPY_ANY = os.environ.get("BAHDANAU_COPY_ANY", "1") == "1"
SC_BUFS = int(os.environ.get("BAHDANAU_SC_BUFS", "1"))

F32 = mybir.dt.float32
F32R = mybir.dt.float32r
BF16 = mybir.dt.bfloat16
AF = mybir.ActivationFunctionType

# "f32r": fp32-width operands in replicated-fp32 PE mode (full PE rate).
# "bf16": main (values @ W1) matmul in bf16 instead.
KP_MODE = os.environ.get("BAHDANAU_KP_MODE", "f32r")
# Repeat the whole batch loop N times (identical output; for timing the
# marginal cost of the kernel body under a constant-overhead harness).
LOOP_MULT = int(os.environ.get("BAHDANAU_LOOP_MULT", "1"))


def _split_sync_waits(nc, max_waits=1):
    """walrus's CTRL lowering in this toolchain accepts only one sem wait per
    instruction; split any instruction carrying more into preceding bare
    Drain wait-carriers on the same engine."""
    n = 0
    for bb in nc.m.functions[0].blocks:
        insts = bb.instructions
        i = 0
        while i < len(insts):
            inst = insts[i]
            si = inst.sync_info
            if si is not None and si.on_wait and len(si.on_wait) > max_waits:
                waits = list(si.on_wait)
                extra, keep = waits[:-max_waits], waits[-max_waits:]
                carriers = []
                for j in range(0, len(extra), max_waits):
                    c = mybir.InstDrain(name=f"{inst.name}-ws{n}", ins=[], outs=[])
                    n += 1
                    c.engine = inst.engine
                    c.sync_info = mybir.SyncInfo(
                        on_wait=extra[j : j + max_waits], on_update=[]
                    )
                    nc.register_instruction(c, overwrite=True)
                    carriers.append(c)
                si.on_wait = keep
                insts[i:i] = carriers
                i += len(carriers)
            i += 1
    return n


def _kernel_body(ctx, tc, q, v, w1, b1, w2, b2, vv, ctx_o, aw_o):
    nc = tc.nc
    kp_bf16 = KP_MODE == "bf16"

    const = ctx.enter_context(tc.tile_pool(name="const", bufs=1))
    identity = const.tile([P, P], F32)
    make_identity(nc, identity[:])
    identr = const.tile([P, P], F32R)
    nc.vector.tensor_copy(out=identr[:], in_=identity[:])
    ones = const.tile([1, 1], F32)
    nc.vector.memset(ones[:], 1.0)
    ones128 = const.tile([1, P], F32)
    nc.vector.memset(ones128[:], 1.0)

    # V / b1 / b2 are loaded as contiguous rows and spread onto partitions via
    # K=1 matmuls against `ones` (strided 4-byte-element DMA is descriptor
    # bound).  Column i of the result is the u-tile-i slice.
    vsb = const.tile([P, UT], F32R)  # V in [u mod 128, u-tile] layout
    bias12 = const.tile([P, UT], F32)  # b1 + b2
    with (
        tc.tile_pool(name="rowstage", bufs=1) as rows,
        tc.tile_pool(name="rowps", bufs=2, space="PSUM") as rowps,
    ):
        vrow = rows.tile([1, U], F32)
        b1row = rows.tile([1, U], F32)
        b2row = rows.tile([1, U], F32)
        nc.sync.dma_start(out=vrow[:], in_=vv.rearrange("u o -> (o u)")[None, :])
        nc.sync.dma_start(out=b1row[:], in_=b1[None, :])
        nc.sync.dma_start(out=b2row[:], in_=b2[None, :])
        nc.vector.tensor_add(b1row[:], b1row[:], b2row[:])
        pv = rowps.tile([P, UT], F32)
        pb = rowps.tile([P, UT], F32)
        for i in range(UT):
            nc.tensor.matmul(
                pv[:, i : i + 1], lhsT=vrow[:, i * P : (i + 1) * P], rhs=ones[:],
                start=True, stop=True, skip_group_check=True,
            )
            nc.tensor.matmul(
                pb[:, i : i + 1], lhsT=b1row[:, i * P : (i + 1) * P], rhs=ones[:],
                start=True, stop=True, skip_group_check=True,
            )
        nc.vector.tensor_copy(out=vsb[:], in_=pv[:])
        nc.vector.tensor_copy(out=bias12[:], in_=pb[:])

    # qp[u, i, b] = (query @ W2 + b2 + b1) per u-tile, per-partition bias for tanh
    qp = const.tile([P, UT, BPC], F32)

    # ---- W1 resident for the whole kernel ----
    w1_dt = BF16 if kp_bf16 else F32R
    w1p = ctx.enter_context(tc.tile_pool(name="w1", bufs=1))
    w1t = w1p.tile([P, DT, U], w1_dt)

    if kp_bf16:
        with tc.tile_pool(name="w1stage", bufs=2) as w1s:
            for j in range(DT):
                stage = w1s.tile([P, U], F32)
                nc.sync.dma_start(out=stage[:], in_=w1[j * P : (j + 1) * P, :])
                nc.vector.tensor_copy(out=w1t[:, j, :], in_=stage[:])
    else:
        for j in range(DT):
            nc.sync.dma_start(out=w1t[:, j, :], in_=w1[j * P : (j + 1) * P, :])

    # ---- query projection phase (scoped pools; W2 released afterwards) ----
    with (
        tc.tile_pool(name="qphase", bufs=1) as qpool,
        tc.tile_pool(name="qps", bufs=2, space="PSUM") as qps,
    ):
        qnat = qpool.tile([BPC, D], F32R)
        nc.sync.dma_start(out=qnat[:], in_=q[:])
        qT = qpool.tile([P, DT, BPC], F32R)
        for j in range(DT):
            pq = qps.tile([P, BPC], F32R)
            nc.tensor.transpose(
                pq[:], qnat[:, j * P : (j + 1) * P], identr[:BPC, :BPC]
            )
            nc.vector.tensor_copy(out=qT[:, j, :], in_=pq[:])

        w2t = qpool.tile([P, DT, U], F32R)
        for j in range(DT):
            nc.sync.dma_start(out=w2t[:, j, :], in_=w2[j * P : (j + 1) * P, :])

        qpT = qpool.tile([BPC, U], F32)
        for h in range(U // 512):
            pqp = qps.tile([BPC, 512], F32)
            for j in range(DT):
                nc.tensor.matmul(
                    pqp[:],
                    lhsT=qT[:, j, :],
                    rhs=w2t[:, j, h * 512 : (h + 1) * 512],
                    start=(j == 0),
                    stop=(j == DT - 1),
                )
            nc.vector.tensor_copy(out=qpT[:, h * 512 : (h + 1) * 512], in_=pqp[:])

        for i in range(UT):
            pq2 = qps.tile([P, BPC], F32)
            nc.tensor.transpose(
                pq2[:], qpT[:, i * P : (i + 1) * P], identity[:BPC, :BPC]
            )
            nc.scalar.activation(
                out=qp[:, i, :],
                in_=pq2[:],
                func=AF.Identity,
                bias=bias12[:, i : i + 1],
                scale=1.0,
            )

    # ---- main pools ----
    vt_dt = BF16 if kp_bf16 else F32R
    natp = ctx.enter_context(tc.tile_pool(name="nat", bufs=NAT_BUFS))
    vtp = ctx.enter_context(tc.tile_pool(name="vt", bufs=VT_BUFS))
    thp = ctx.enter_context(tc.tile_pool(name="th", bufs=TH_BUFS))
    scp = ctx.enter_context(tc.tile_pool(name="scores", bufs=SC_BUFS))
    awp = ctx.enter_context(tc.tile_pool(name="aw", bufs=2))
    ctxp = ctx.enter_context(tc.tile_pool(name="ctxacc", bufs=2))
    misc = ctx.enter_context(tc.tile_pool(name="misc", bufs=4))

    trps = ctx.enter_context(tc.tile_pool(name="trps", bufs=2, space="PSUM"))
    kpps = ctx.enter_context(tc.tile_pool(name="kpps", bufs=KP_BUFS, space="PSUM"))
    scps = ctx.enter_context(tc.tile_pool(name="scps", bufs=1, space="PSUM"))
    bcps = ctx.enter_context(tc.tile_pool(name="bcps", bufs=1, space="PSUM"))
    ctps = ctx.enter_context(tc.tile_pool(name="ctps", bufs=2, space="PSUM"))

    alu = mybir.AluOpType

    for b in [bi for _ in range(LOOP_MULT) for bi in range(BPC)]:
        scores = scp.tile([1, T], F32)
        # online-softmax running state on partition 0: negm = -max, s, ctx row
        negm = misc.tile([1, 1], F32)
        nc.vector.memset(negm[:], 1e30)
        s_run = misc.tile([1, 1], F32)
        nc.vector.memset(s_run[:], 0.0)
        ctx_acc = ctxp.tile([1, D], F32, tag="ctx")
        nc.vector.memset(ctx_acc[:], 0.0)

        for ch in range(NCH):
            chunk_nats = []
            for tt in range(TT):
                nat = natp.tile([P, D], F32R)
                t0 = (ch * TT + tt) * P
                if b == 0 and ch == 0:
                    # jump the very first chunk ahead of the 17 MB of weight
                    # DMA so the PE pipeline primes while weights stream
                    with tc.high_priority():
                        nc.sync.dma_start(out=nat[:], in_=v[b, t0 : t0 + P, :])
                else:
                    nc.sync.dma_start(out=nat[:], in_=v[b, t0 : t0 + P, :])
                chunk_nats.append(nat)

            # transpose the chunk: vT[d=128, j, t=TCH]
            vT = vtp.tile([P, DT, TCH], vt_dt)
            for j in range(DT):
                pt = trps.tile([P, TCH], F32R)
                for tt in range(TT):
                    nc.tensor.transpose(
                        pt[:, tt * P : (tt + 1) * P],
                        chunk_nats[tt][:, j * P : (j + 1) * P],
                        identr[:],
                    )
                if COPY_ANY:
                    nc.any.tensor_copy(out=vT[:, j, :], in_=pt[:])
                else:
                    nc.vector.tensor_copy(out=vT[:, j, :], in_=pt[:])

            # kp = W1.T @ vT per u-tile; tanh(+bias); score accumulation
            sc = scps.tile([1, TCH], F32)
            for i in range(UT):
                kp = kpps.tile([P, TCH], F32)
                for j in range(DT):
                    nc.tensor.matmul(
                        kp[:],
                        lhsT=w1t[:, j, i * P : (i + 1) * P],
                        rhs=vT[:, j, :],
                        start=(j == 0),
                        stop=(j == DT - 1),
                    )
                th = thp.tile([P, TCH], F32R)
                nc.scalar.activation(
                    out=th[:],
                    in_=kp[:],
                    func=AF.Tanh,
                    bias=qp[:, i, b : b + 1],
                    scale=1.0,
                )
                nc.tensor.matmul(
                    sc[:],
                    lhsT=vsb[:, i : i + 1],
                    rhs=th[:],
                    start=(i == 0),
                    stop=(i == UT - 1),
                    skip_group_check=True,
                )
            sc_sb = scores[:, ch * TCH : (ch + 1) * TCH]
            nc.vector.tensor_copy(out=sc_sb, in_=sc[:])

            # ---- online softmax state update ----
            ncmax = misc.tile([1, 1], F32)
            nc.vector.tensor_reduce(
                out=ncmax[:], in_=sc_sb, axis=mybir.AxisListType.X,
                op=alu.max, negate=True,
            )
            negm_new = misc.tile([1, 1], F32)
            nc.vector.tensor_tensor(
                out=negm_new[:], in0=negm[:], in1=ncmax[:], op=alu.min
            )
            dneg = misc.tile([1, 1], F32)
            nc.vector.tensor_sub(dneg[:], negm_new[:], negm[:])
            alpha = misc.tile([1, 1], F32)
            nc.scalar.activation(out=alpha[:], in_=dneg[:], func=AF.Exp)
            wrow = awp.tile([1, TCH], F32)
            csum = misc.tile([1, 1], F32)
            nc.scalar.activation(
                out=wrow[:], in_=sc_sb, func=AF.Exp, bias=negm_new[:], scale=1.0,
                accum_out=csum[:],
            )
            s_new = misc.tile([1, 1], F32)
            nc.vector.tensor_scalar(
                out=s_new[:], in0=s_run[:], scalar1=alpha[:], scalar2=csum[:],
                op0=alu.mult, op1=alu.add,
            )

            # ---- context update: ctx_new = alpha * ctx_acc + wrow @ nats ----
            pwt = bcps.tile([P, TT], F32, tag="bc")
            for c in range(TT):
                nc.tensor.matmul(
                    pwt[:, c : c + 1], lhsT=wrow[:, c * P : (c + 1) * P],
                    rhs=ones[:], start=True, stop=True, skip_group_check=True,
                )
            wcol = misc.tile([P, TT], F32R)
            nc.vector.tensor_copy(out=wcol[:], in_=pwt[:])
            ctx_new = ctxp.tile([1, D], F32, tag="ctx")
            for h in range(D // DCH):
                pc = ctps.tile([1, DCH], F32)
                for c in range(TT):
                    nc.tensor.matmul(
                        pc[:],
                        lhsT=wcol[:, c : c + 1],
                        rhs=chunk_nats[c][:, h * DCH : (h + 1) * DCH],
                        start=(c == 0),
                        stop=(c == TT - 1),
                    )
                sl = slice(h * DCH, (h + 1) * DCH)
                nc.vector.tensor_scalar_mul(
                    ctx_new[:, sl], ctx_acc[:, sl], alpha[:]
                )
                nc.vector.tensor_add(ctx_new[:, sl], ctx_new[:, sl], pc[:])
            negm, s_run, ctx_acc = negm_new, s_new, ctx_new

        # ---- batch epilogue: normalize ctx, emit attention weights ----
        invs = misc.tile([1, 1], F32)
        nc.vector.reciprocal(out=invs[:], in_=s_run[:])
        ctxf = ctxp.tile([1, D], F32, tag="ctx")
        nc.vector.tensor_scalar_mul(ctxf[:], ctx_acc[:], invs[:])
        nc.sync.dma_start(out=ctx_o[b : b + 1, :], in_=ctxf[:])

        e = awp.tile([1, T], F32)
        nc.scalar.activation(
            out=e[:], in_=scores[:], func=AF.Exp, bias=negm[:], scale=1.0
        )
        nc.vector.tensor_scalar_mul(e[:], e[:], invs[:])
        nc.sync.dma_start(out=aw_o[b : b + 1, :], in_=e[:])


def build_kernel():
    nc = bass.Bass("TRN2", target_bir_lowering=False, debug=False)
    q = nc.dram_tensor("query", [BPC, D], F32R, kind="ExternalInput").ap()
    v = nc.dram_tensor("values", [BPC, T, D], F32R, kind="ExternalInput").ap()
    w1 = nc.dram_tensor("W1", [D, U], F32R, kind="ExternalInput").ap()
    b1 = nc.dram_tensor("b1", [U], F32, kind="ExternalInput").ap()
    w2 = nc.dram_tensor("W2", [D, U], F32R, kind="ExternalInput").ap()
    b2 = nc.dram_tensor("b2", [U], F32, kind="ExternalInput").ap()
    vv = nc.dram_tensor("V", [U, 1], F32, kind="ExternalInput").ap()
    ctx_o = nc.dram_tensor("ctx", [BPC, D], F32, kind="ExternalOutput").ap()
    aw_o = nc.dram_tensor("aw", [BPC, T], F32, kind="ExternalOutput").ap()

    with tile.TileContext(nc) as tc:
        with ExitStack() as ctx:
            _kernel_body(ctx, tc, q, v, w1, b1, w2, b2, vv, ctx_o, aw_o)
    _split_sync_waits(nc)
    return nc


_NC_CACHE = None
_RUNNER_CACHE = None


def _get_runner():
    """Build the sharded 8-core executable once; reuse across kernel() calls."""
    global _NC_CACHE, _RUNNER_CACHE
    if _RUNNER_CACHE is not None:
        return _RUNNER_CACHE
    import jax
    from jax.experimental.shard_map import shard_map
    from jax.sharding import Mesh, NamedSharding, PartitionSpec

    from concourse import bass2jax
    from concourse.bass2jax import _bass_exec_p, install_neuronx_cc_hook

    if _NC_CACHE is None:
        _NC_CACHE = build_kernel()
    nc = _NC_CACHE
    install_neuronx_cc_hook()
    partition_name = nc.partition_id_tensor.name if nc.partition_id_tensor else None

    in_names, out_names, out_avals, zero_outs = [], [], [], []
    for alloc in nc.m.functions[0].allocations:
        if not isinstance(alloc, mybir.MemoryLocationSet):
            continue
        name = alloc.memorylocations[0].name
        if alloc.kind == "ExternalInput":
            if name != partition_name:
                in_names.append(name)
        elif alloc.kind == "ExternalOutput":
            out_names.append(name)
            shape = tuple(alloc.tensor_shape)
            dtype = mybir.dt.np(alloc.dtype)
            out_avals.append(jax.core.ShapedArray(shape, dtype))
            zero_outs.append(np.zeros(shape, dtype))
    n_params = len(in_names)
    all_in_names = list(in_names) + list(out_names)
    if partition_name is not None:
        all_in_names.append(partition_name)
    donate = tuple(range(n_params, n_params + len(out_names)))

    def _body(*args):
        operands = list(args)
        if partition_name is not None:
            operands.append(bass2jax.partition_id_tensor())
        outs = _bass_exec_p.bind(
            *operands,
            out_avals=tuple(out_avals),
            in_names=tuple(all_in_names),
            out_names=tuple(out_names),
            lowering_input_output_aliases=(),
            sim_require_finite=True,
            sim_require_nnan=True,
            nc=nc,
        )
        return tuple(outs)

    devices = jax.devices()[:NCORES]
    mesh = Mesh(np.asarray(devices), ("core",))
    fn = jax.jit(
        shard_map(
            _body,
            mesh=mesh,
            in_specs=(PartitionSpec("core"),) * (n_params + len(out_names)),
            out_specs=(PartitionSpec("core"),) * len(out_names),
            check_rep=False,
        ),
        donate_argnums=donate,
        keep_unused=True,
    )
    sharding = NamedSharding(mesh, PartitionSpec("core"))
    _RUNNER_CACHE = (fn, in_names, out_names, out_avals, zero_outs, sharding)
    return _RUNNER_CACHE


def _run_cached(in_maps):
    import jax

    fn, in_names, out_names, out_avals, zero_outs, sharding = _get_runner()
    n_cores = len(in_maps)
    concat_in = [
        jax.device_put(
            np.concatenate([np.asarray(in_maps[c][n]) for c in range(n_cores)], axis=0),
            sharding,
        )
        for n in in_names
    ]
    zeros = [
        jax.device_put(np.zeros((n_cores * z.shape[0], *z.shape[1:]), z.dtype), sharding)
        for z in zero_outs
    ]
    outs = fn(*concat_in, *zeros)
    return [
        {
            name: np.asarray(outs[i]).reshape(n_cores, *out_avals[i].shape)[c]
            for i, name in enumerate(out_names)
        }
        for c in range(n_cores)
    ]


def kernel(query, values, W1, b1, W2, b2, V, bv):
    global _NC_CACHE
    query = np.ascontiguousarray(np.asarray(query, dtype=np.float32))
    values = np.ascontiguousarray(np.asarray(values, dtype=np.float32))
    W1 = np.ascontiguousarray(np.asarray(W1, dtype=np.float32))
    b1 = np.ascontiguousarray(np.asarray(b1, dtype=np.float32))
    W2 = np.ascontiguousarray(np.asarray(W2, dtype=np.float32))
    b2 = np.ascontiguousarray(np.asarray(b2, dtype=np.float32))
    V = np.ascontiguousarray(np.asarray(V, dtype=np.float32))

    core_ids = list(range(NCORES))
    in_maps = []
    for c in core_ids:
        sl = slice(c * BPC, (c + 1) * BPC)
        in_maps.append(
            {
                "query": query[sl],
                "values": values[sl],
                "W1": W1,
                "b1": b1,
                "W2": W2,
                "b2": b2,
                "V": V,
            }
        )
    try:
        results = _run_cached(in_maps)
    except Exception:
        if _NC_CACHE is None:
            _NC_CACHE = build_kernel()
        results = run_bass_kernel_spmd(_NC_CACHE, in_maps, core_ids).results
    ctx_full = np.concatenate([results[c]["ctx"] for c in core_ids], axis=0)
    aw_full = np.concatenate([results[c]["aw"] for c in core_ids], axis=0)
    return ctx_full.astype(np.float32), aw_full[:, :, None].astype(np.float32)


# revision 24
# speedup vs baseline: 1.0225x; 1.0225x over previous
"""Bahdanau attention Trainium2 kernel.

  keys_proj = values @ W1 + b1            # (B, T, U)
  query_proj = query @ W2 + b2            # (B, 1, U)
  score = tanh(keys_proj + query_proj) @ V + bv   # (B, T, 1)
  aw = softmax(score, axis=1)             # (B, T, 1)
  ctx = sum(aw * values, axis=1)          # (B, D)

Sharding: data-parallel over batch, 4 batches per core on 8 cores. Each core
is fully independent (no collectives).

Per-core dataflow (streaming over t-chunks with an online softmax, so values
is read from HBM exactly once and never retained beyond the current chunk):
  - values chunk loaded naturally [t=128, d=2048] (contiguous DMA).
  - PE-transpose 128x128 blocks -> vT [d=128, t=TCH] (f32r, 1.5 cyc/row).
  - kp[u=128, t] += W1[d,u-tile].T @ vT (PSUM accumulate over 16 d-tiles),
    float32r so the PE runs at full rate with 4-byte operands (moving
    dim >= 256).  float32r is a reduced-mantissa fp32 PE mode; end-to-end
    error vs the fp32 reference measures ~2.4e-4.
  - ScalarE: th = tanh(kp + (q @ W2 + b2 + b1)[u]) with per-partition bias,
    PSUM -> SBUF in one activation op.
  - score[1, t] += V[u-tile].T @ th  (PSUM accumulate over 8 u-tiles).
  - online softmax state update on partition 0 (negm = -running max, s =
    running sum): DVE reduce max (negated) / min / sub, ACT exp of the
    chunk with fused accum_out sum.  bv is dropped: softmax is
    shift-invariant and both outputs depend on score only through softmax.
  - context update ctx[1, d] = alpha * ctx + w_chunk @ values_chunk: chunk
    weights spread to [t=128, TT] columns via K=1 matmuls against ones,
    then TT accumulating matmuls against the still-resident natural chunk
    tiles, and a DVE rescale-add.
  - batch epilogue: ctx/s DMA'd out, attention weights recomputed from the
    stored score row as exp(score - max)/s.
"""

import os
from contextlib import ExitStack

import numpy as np

import concourse.bass as bass
import concourse.mybir as mybir
import concourse.tile as tile
from concourse.bass_utils import run_bass_kernel_spmd
from concourse.masks import make_identity

B, T, D, U = 32, 1024, 2048, 1024
NCORES = 8
BPC = B // NCORES  # batches per core
P = 128
DT = D // P  # 16 d-tiles
UT = U // P  # 8 u-tiles
TCH = int(os.environ.get("BAHDANAU_TCH", "512"))  # main-matmul moving dim; >=256 for f32r full rate
NCH = T // TCH  # chunks per batch
TT = TCH // P  # t-subtiles per chunk
DCH = 512  # d-chunk of the context matmul
NAT_BUFS = int(os.environ.get("BAHDANAU_NAT_BUFS", "8"))
VT_BUFS = int(os.environ.get("BAHDANAU_VT_BUFS", "1"))
TH_BUFS = int(os.environ.get("BAHDANAU_TH_BUFS", "2"))
KP_BUFS = int(os.environ.get("BAHDANAU_KP_BUFS", "2"))
COPY_ANY = os.environ.get("BAHDANAU_COPY_ANY", "1") == "1"
SC_BUFS = int(os.environ.get("BAHDANAU_SC_BUFS", "1"))

F32 = mybir.dt.float32
F32R = mybir.dt.float32r
BF16 = mybir.dt.bfloat16
AF = mybir.ActivationFunctionType

# "f32r": fp32-width operands in replicated-fp32 PE mode (full PE rate).
# "bf16": main (values @ W1) matmul in bf16 instead.
KP_MODE = os.environ.get("BAHDANAU_KP_MODE", "f32r")
# Repeat the whole batch loop N times (identical output; for timing the
# marginal cost of the kernel body under a constant-overhead harness).
LOOP_MULT = int(os.environ.get("BAHDANAU_LOOP_MULT", "1"))


def _split_sync_waits(nc, max_waits=1):
    """walrus's CTRL lowering in this toolchain accepts only one sem wait per
    instruction; split any instruction carrying more into preceding bare
    Drain wait-carriers on the same engine."""
    n = 0
    for bb in nc.m.functions[0].blocks:
        insts = bb.instructions
        i = 0
        while i < len(insts):
            inst = insts[i]
            si = inst.sync_info
            if si is not None and si.on_wait and len(si.on_wait) > max_waits:
                waits = list(si.on_wait)
                extra, keep = waits[:-max_waits], waits[-max_waits:]
                carriers = []
                for j in range(0, len(extra), max_waits):
                    c = mybir.InstDrain(name=f"{inst.name}-ws{n}", ins=[], outs=[])
                    n += 1
                    c.engine = inst.engine
                    c.sync_info = mybir.SyncInfo(
                        on_wait=extra[j : j + max_waits], on_update=[]
                    )
                    nc.register_instruction(c, overwrite=True)
                    carriers.append(c)
                si.on_wait = keep
                insts[i:i] = carriers
                i += len(carriers)
            i += 1
    return n


def _kernel_body(ctx, tc, q, v, w1, b1, w2, b2, vv, ctx_o, aw_o):
    nc = tc.nc
    kp_bf16 = KP_MODE == "bf16"

    const = ctx.enter_context(tc.tile_pool(name="const", bufs=1))
    identity = const.tile([P, P], F32)
    make_identity(nc, identity[:])
    identr = const.tile([P, P], F32R)
    nc.vector.tensor_copy(out=identr[:], in_=identity[:])
    ones = const.tile([1, 1], F32)
    nc.vector.memset(ones[:], 1.0)
    ones128 = const.tile([1, P], F32)
    nc.vector.memset(ones128[:], 1.0)

    # V / b1 / b2 are loaded as contiguous rows and spread onto partitions via
    # K=1 matmuls against `ones` (strided 4-byte-element DMA is descriptor
    # bound).  Column i of the result is the u-tile-i slice.
    vsb = const.tile([P, UT], F32R)  # V in [u mod 128, u-tile] layout
    bias12 = const.tile([P, UT], F32)  # b1 + b2
    with (
        tc.tile_pool(name="rowstage", bufs=1) as rows,
        tc.tile_pool(name="rowps", bufs=2, space="PSUM") as rowps,
    ):
        vrow = rows.tile([1, U], F32)
        b1row = rows.tile([1, U], F32)
        b2row = rows.tile([1, U], F32)
        nc.sync.dma_start(out=vrow[:], in_=vv.rearrange("u o -> (o u)")[None, :])
        nc.sync.dma_start(out=b1row[:], in_=b1[None, :])
        nc.sync.dma_start(out=b2row[:], in_=b2[None, :])
        nc.vector.tensor_add(b1row[:], b1row[:], b2row[:])
        pv = rowps.tile([P, UT], F32)
        pb = rowps.tile([P, UT], F32)
        for i in range(UT):
            nc.tensor.matmul(
                pv[:, i : i + 1], lhsT=vrow[:, i * P : (i + 1) * P], rhs=ones[:],
                start=True, stop=True, skip_group_check=True,
            )
            nc.tensor.matmul(
                pb[:, i : i + 1], lhsT=b1row[:, i * P : (i + 1) * P], rhs=ones[:],
                start=True, stop=True, skip_group_check=True,
            )
        nc.vector.tensor_copy(out=vsb[:], in_=pv[:])
        nc.vector.tensor_copy(out=bias12[:], in_=pb[:])

    # qp[u, i, b] = (query @ W2 + b2 + b1) per u-tile, per-partition bias for tanh
    qp = const.tile([P, UT, BPC], F32)

    # ---- values stream pool + pre-issued first chunks ----
    # All DMAs drain in SP program order, so the first values chunks must be
    # issued BEFORE the 17 MB of W1/W2 weight DMA or the PE pipeline idles
    # ~50 us at startup waiting for its first vT chunk.
    natp = ctx.enter_context(tc.tile_pool(name="nat", bufs=NAT_BUFS))
    pre_nats = []
    for tt0 in range(min(2 * TT, T // P)):
        nat = natp.tile([P, D], F32R)
        nc.sync.dma_start(out=nat[:], in_=v[0, tt0 * P : (tt0 + 1) * P, :])
        pre_nats.append(nat)

    # ---- W1 resident for the whole kernel ----
    w1_dt = BF16 if kp_bf16 else F32R
    w1p = ctx.enter_context(tc.tile_pool(name="w1", bufs=1))
    w1t = w1p.tile([P, DT, U], w1_dt)

    if kp_bf16:
        with tc.tile_pool(name="w1stage", bufs=2) as w1s:
            for j in range(DT):
                stage = w1s.tile([P, U], F32)
                nc.sync.dma_start(out=stage[:], in_=w1[j * P : (j + 1) * P, :])
                nc.vector.tensor_copy(out=w1t[:, j, :], in_=stage[:])
    else:
        for j in range(DT):
            nc.sync.dma_start(out=w1t[:, j, :], in_=w1[j * P : (j + 1) * P, :])

    # ---- query projection phase (scoped pools; W2 released afterwards) ----
    with (
        tc.tile_pool(name="qphase", bufs=1) as qpool,
        tc.tile_pool(name="qps", bufs=2, space="PSUM") as qps,
    ):
        qnat = qpool.tile([BPC, D], F32R)
        nc.sync.dma_start(out=qnat[:], in_=q[:])
        qT = qpool.tile([P, DT, BPC], F32R)
        for j in range(DT):
            pq = qps.tile([P, BPC], F32R)
            nc.tensor.transpose(
                pq[:], qnat[:, j * P : (j + 1) * P], identr[:BPC, :BPC]
            )
            nc.vector.tensor_copy(out=qT[:, j, :], in_=pq[:])

        w2t = qpool.tile([P, DT, U], F32R)
        for j in range(DT):
            nc.sync.dma_start(out=w2t[:, j, :], in_=w2[j * P : (j + 1) * P, :])

        qpT = qpool.tile([BPC, U], F32)
        for h in range(U // 512):
            pqp = qps.tile([BPC, 512], F32)
            for j in range(DT):
                nc.tensor.matmul(
                    pqp[:],
                    lhsT=qT[:, j, :],
                    rhs=w2t[:, j, h * 512 : (h + 1) * 512],
                    start=(j == 0),
                    stop=(j == DT - 1),
                )
            nc.vector.tensor_copy(out=qpT[:, h * 512 : (h + 1) * 512], in_=pqp[:])

        for i in range(UT):
            pq2 = qps.tile([P, BPC], F32)
            nc.tensor.transpose(
                pq2[:], qpT[:, i * P : (i + 1) * P], identity[:BPC, :BPC]
            )
            nc.scalar.activation(
                out=qp[:, i, :],
                in_=pq2[:],
                func=AF.Identity,
                bias=bias12[:, i : i + 1],
                scale=1.0,
            )

    # ---- main pools ----
    vt_dt = BF16 if kp_bf16 else F32R
    vtp = ctx.enter_context(tc.tile_pool(name="vt", bufs=VT_BUFS))
    thp = ctx.enter_context(tc.tile_pool(name="th", bufs=TH_BUFS))
    scp = ctx.enter_context(tc.tile_pool(name="scores", bufs=SC_BUFS))
    awp = ctx.enter_context(tc.tile_pool(name="aw", bufs=2))
    ctxp = ctx.enter_context(tc.tile_pool(name="ctxacc", bufs=2))
    misc = ctx.enter_context(tc.tile_pool(name="misc", bufs=4))

    trps = ctx.enter_context(tc.tile_pool(name="trps", bufs=2, space="PSUM"))
    kpps = ctx.enter_context(tc.tile_pool(name="kpps", bufs=KP_BUFS, space="PSUM"))
    scps = ctx.enter_context(tc.tile_pool(name="scps", bufs=1, space="PSUM"))
    bcps = ctx.enter_context(tc.tile_pool(name="bcps", bufs=1, space="PSUM"))
    ctps = ctx.enter_context(tc.tile_pool(name="ctps", bufs=2, space="PSUM"))

    alu = mybir.AluOpType

    first_iter = True
    for b in [bi for _ in range(LOOP_MULT) for bi in range(BPC)]:
        scores = scp.tile([1, T], F32)
        # online-softmax running state on partition 0: negm = -max, s, ctx row
        negm = misc.tile([1, 1], F32)
        nc.vector.memset(negm[:], 1e30)
        s_run = misc.tile([1, 1], F32)
        nc.vector.memset(s_run[:], 0.0)
        ctx_acc = ctxp.tile([1, D], F32, tag="ctx")
        nc.vector.memset(ctx_acc[:], 0.0)

        for ch in range(NCH):
            chunk_nats = []
            for tt in range(TT):
                t0 = (ch * TT + tt) * P
                if first_iter and t0 // P < len(pre_nats):
                    chunk_nats.append(pre_nats[t0 // P])
                    continue
                nat = natp.tile([P, D], F32R)
                nc.sync.dma_start(out=nat[:], in_=v[b, t0 : t0 + P, :])
                chunk_nats.append(nat)

            # transpose the chunk: vT[d=128, j, t=TCH]
            vT = vtp.tile([P, DT, TCH], vt_dt)
            for j in range(DT):
                pt = trps.tile([P, TCH], F32R)
                for tt in range(TT):
                    nc.tensor.transpose(
                        pt[:, tt * P : (tt + 1) * P],
                        chunk_nats[tt][:, j * P : (j + 1) * P],
                        identr[:],
                    )
                if COPY_ANY:
                    nc.any.tensor_copy(out=vT[:, j, :], in_=pt[:])
                else:
                    nc.vector.tensor_copy(out=vT[:, j, :], in_=pt[:])

            # kp = W1.T @ vT per u-tile; tanh(+bias); score accumulation
            sc = scps.tile([1, TCH], F32)
            for i in range(UT):
                kp = kpps.tile([P, TCH], F32)
                for j in range(DT):
                    nc.tensor.matmul(
                        kp[:],
                        lhsT=w1t[:, j, i * P : (i + 1) * P],
                        rhs=vT[:, j, :],
                        start=(j == 0),
                        stop=(j == DT - 1),
                    )
                th = thp.tile([P, TCH], F32R)
                nc.scalar.activation(
                    out=th[:],
                    in_=kp[:],
                    func=AF.Tanh,
                    bias=qp[:, i, b : b + 1],
                    scale=1.0,
                )
                nc.tensor.matmul(
                    sc[:],
                    lhsT=vsb[:, i : i + 1],
                    rhs=th[:],
                    start=(i == 0),
                    stop=(i == UT - 1),
                    skip_group_check=True,
                )
            sc_sb = scores[:, ch * TCH : (ch + 1) * TCH]
            nc.vector.tensor_copy(out=sc_sb, in_=sc[:])

            # ---- online softmax state update ----
            ncmax = misc.tile([1, 1], F32)
            nc.vector.tensor_reduce(
                out=ncmax[:], in_=sc_sb, axis=mybir.AxisListType.X,
                op=alu.max, negate=True,
            )
            negm_new = misc.tile([1, 1], F32)
            nc.vector.tensor_tensor(
                out=negm_new[:], in0=negm[:], in1=ncmax[:], op=alu.min
            )
            dneg = misc.tile([1, 1], F32)
            nc.vector.tensor_sub(dneg[:], negm_new[:], negm[:])
            alpha = misc.tile([1, 1], F32)
            nc.scalar.activation(out=alpha[:], in_=dneg[:], func=AF.Exp)
            wrow = awp.tile([1, TCH], F32)
            csum = misc.tile([1, 1], F32)
            nc.scalar.activation(
                out=wrow[:], in_=sc_sb, func=AF.Exp, bias=negm_new[:], scale=1.0,
                accum_out=csum[:],
            )
            s_new = misc.tile([1, 1], F32)
            nc.vector.tensor_scalar(
                out=s_new[:], in0=s_run[:], scalar1=alpha[:], scalar2=csum[:],
                op0=alu.mult, op1=alu.add,
            )

            # ---- context update: ctx_new = alpha * ctx_acc + wrow @ nats ----
            pwt = bcps.tile([P, TT], F32, tag="bc")
            for c in range(TT):
                nc.tensor.matmul(
                    pwt[:, c : c + 1], lhsT=wrow[:, c * P : (c + 1) * P],
                    rhs=ones[:], start=True, stop=True, skip_group_check=True,
                )
            wcol = misc.tile([P, TT], F32R)
            nc.vector.tensor_copy(out=wcol[:], in_=pwt[:])
            ctx_new = ctxp.tile([1, D], F32, tag="ctx")
            for h in range(D // DCH):
                pc = ctps.tile([1, DCH], F32)
                for c in range(TT):
                    nc.tensor.matmul(
                        pc[:],
                        lhsT=wcol[:, c : c + 1],
                        rhs=chunk_nats[c][:, h * DCH : (h + 1) * DCH],
                        start=(c == 0),
                        stop=(c == TT - 1),
                    )
                sl = slice(h * DCH, (h + 1) * DCH)
                nc.vector.tensor_scalar_mul(
                    ctx_new[:, sl], ctx_acc[:, sl], alpha[:]
                )
                nc.vector.tensor_add(ctx_new[:, sl], ctx_new[:, sl], pc[:])
            negm, s_run, ctx_acc = negm_new, s_new, ctx_new
        first_iter = False

        # ---- batch epilogue: normalize ctx, emit attention weights ----
        invs = misc.tile([1, 1], F32)
        nc.vector.reciprocal(out=invs[:], in_=s_run[:])
        ctxf = ctxp.tile([1, D], F32, tag="ctx")
        nc.vector.tensor_scalar_mul(ctxf[:], ctx_acc[:], invs[:])
        nc.sync.dma_start(out=ctx_o[b : b + 1, :], in_=ctxf[:])

        e = awp.tile([1, T], F32)
        nc.scalar.activation(
            out=e[:], in_=scores[:], func=AF.Exp, bias=negm[:], scale=1.0
        )
        nc.vector.tensor_scalar_mul(e[:], e[:], invs[:])
        nc.sync.dma_start(out=aw_o[b : b + 1, :], in_=e[:])


def build_kernel():
    nc = bass.Bass("TRN2", target_bir_lowering=False, debug=False)
    q = nc.dram_tensor("query", [BPC, D], F32R, kind="ExternalInput").ap()
    v = nc.dram_tensor("values", [BPC, T, D], F32R, kind="ExternalInput").ap()
    w1 = nc.dram_tensor("W1", [D, U], F32R, kind="ExternalInput").ap()
    b1 = nc.dram_tensor("b1", [U], F32, kind="ExternalInput").ap()
    w2 = nc.dram_tensor("W2", [D, U], F32R, kind="ExternalInput").ap()
    b2 = nc.dram_tensor("b2", [U], F32, kind="ExternalInput").ap()
    vv = nc.dram_tensor("V", [U, 1], F32, kind="ExternalInput").ap()
    ctx_o = nc.dram_tensor("ctx", [BPC, D], F32, kind="ExternalOutput").ap()
    aw_o = nc.dram_tensor("aw", [BPC, T], F32, kind="ExternalOutput").ap()

    with tile.TileContext(nc) as tc:
        with ExitStack() as ctx:
            _kernel_body(ctx, tc, q, v, w1, b1, w2, b2, vv, ctx_o, aw_o)
    _split_sync_waits(nc)
    return nc


_NC_CACHE = None
_RUNNER_CACHE = None


def _get_runner():
    """Build the sharded 8-core executable once; reuse across kernel() calls."""
    global _NC_CACHE, _RUNNER_CACHE
    if _RUNNER_CACHE is not None:
        return _RUNNER_CACHE
    import jax
    from jax.experimental.shard_map import shard_map
    from jax.sharding import Mesh, NamedSharding, PartitionSpec

    from concourse import bass2jax
    from concourse.bass2jax import _bass_exec_p, install_neuronx_cc_hook

    if _NC_CACHE is None:
        _NC_CACHE = build_kernel()
    nc = _NC_CACHE
    install_neuronx_cc_hook()
    partition_name = nc.partition_id_tensor.name if nc.partition_id_tensor else None

    in_names, out_names, out_avals, zero_outs = [], [], [], []
    for alloc in nc.m.functions[0].allocations:
        if not isinstance(alloc, mybir.MemoryLocationSet):
            continue
        name = alloc.memorylocations[0].name
        if alloc.kind == "ExternalInput":
            if name != partition_name:
                in_names.append(name)
        elif alloc.kind == "ExternalOutput":
            out_names.append(name)
            shape = tuple(alloc.tensor_shape)
            dtype = mybir.dt.np(alloc.dtype)
            out_avals.append(jax.core.ShapedArray(shape, dtype))
            zero_outs.append(np.zeros(shape, dtype))
    n_params = len(in_names)
    all_in_names = list(in_names) + list(out_names)
    if partition_name is not None:
        all_in_names.append(partition_name)
    donate = tuple(range(n_params, n_params + len(out_names)))

    def _body(*args):
        operands = list(args)
        if partition_name is not None:
            operands.append(bass2jax.partition_id_tensor())
        outs = _bass_exec_p.bind(
            *operands,
            out_avals=tuple(out_avals),
            in_names=tuple(all_in_names),
            out_names=tuple(out_names),
            lowering_input_output_aliases=(),
            sim_require_finite=True,
            sim_require_nnan=True,
            nc=nc,
        )
        return tuple(outs)

    devices = jax.devices()[:NCORES]
    mesh = Mesh(np.asarray(devices), ("core",))
    fn = jax.jit(
        shard_map(
            _body,
            mesh=mesh,
            in_specs=(PartitionSpec("core"),) * (n_params + len(out_names)),
            out_specs=(PartitionSpec("core"),) * len(out_names),
            check_rep=False,
        ),
        donate_argnums=donate,
        keep_unused=True,
    )
    sharding = NamedSharding(mesh, PartitionSpec("core"))
    _RUNNER_CACHE = (fn, in_names, out_names, out_avals, zero_outs, sharding)
    return _RUNNER_CACHE


def _run_cached(in_maps):
    import jax

    fn, in_names, out_names, out_avals, zero_outs, sharding = _get_runner()
    n_cores = len(in_maps)
    concat_in = [
        jax.device_put(
            np.concatenate([np.asarray(in_maps[c][n]) for c in range(n_cores)], axis=0),
            sharding,
        )
        for n in in_names
    ]
    zeros = [
        jax.device_put(np.zeros((n_cores * z.shape[0], *z.shape[1:]), z.dtype), sharding)
        for z in zero_outs
    ]
    outs = fn(*concat_in, *zeros)
    return [
        {
            name: np.asarray(outs[i]).reshape(n_cores, *out_avals[i].shape)[c]
            for i, name in enumerate(out_names)
        }
        for c in range(n_cores)
    ]


def kernel(query, values, W1, b1, W2, b2, V, bv):
    global _NC_CACHE
    query = np.ascontiguousarray(np.asarray(query, dtype=np.float32))
    values = np.ascontiguousarray(np.asarray(values, dtype=np.float32))
    W1 = np.ascontiguousarray(np.asarray(W1, dtype=np.float32))
    b1 = np.ascontiguousarray(np.asarray(b1, dtype=np.float32))
    W2 = np.ascontiguousarray(np.asarray(W2, dtype=np.float32))
    b2 = np.ascontiguousarray(np.asarray(b2, dtype=np.float32))
    V = np.ascontiguousarray(np.asarray(V, dtype=np.float32))

    core_ids = list(range(NCORES))
    in_maps = []
    for c in core_ids:
        sl = slice(c * BPC, (c + 1) * BPC)
        in_maps.append(
            {
                "query": query[sl],
                "values": values[sl],
                "W1": W1,
                "b1": b1,
                "W2": W2,
                "b2": b2,
                "V": V,
            }
        )
    try:
        results = _run_cached(in_maps)
    except Exception:
        if _NC_CACHE is None:
            _NC_CACHE = build_kernel()
        results = run_bass_kernel_spmd(_NC_CACHE, in_maps, core_ids).results
    ctx_full = np.concatenate([results[c]["ctx"] for c in core_ids], axis=0)
    aw_full = np.concatenate([results[c]["aw"] for c in core_ids], axis=0)
    return ctx_full.astype(np.float32), aw_full[:, :, None].astype(np.float32)
